# revision 20
# baseline (speedup 1.0000x reference)
"""Point spatial attention kernel for Trainium2, data-parallel over batch B=8.

Per core b (owning batch b, x_b [3,4096]):
  stage A: h = W1 @ x_b [64,4096]; local BN sufficient stats via bn_stats/bn_aggr;
           AllReduce#1 of [mean/8, E[h^2]/8] over 8 cores.
  stage B: fold BN1 affine into (s1,t1); h_act = leaky_relu via
           relu(z) - 0.2*relu(-z); feat = W2 @ h_act [128,4096];
           y = Wqkv_pad @ feat [67,4096] (q@0:16 | k@32:48 | v@64:67, padded
           so every SBUF access starts at partition 0/32/64);
           local stats of y; AllReduce#2 of [mean/8, E[y^2]/8].
  stage C: fold BN2 affine (alpha pre-folded into v rows host-side);
           qkv = Relu(s*y + t); DMA k,v to base-0 tiles; build VT_aug tiles
           [128,4] (col 0 = ones for denominator, cols 1:4 = v^T).
  attn:    per 512-col n-chunk: S[m,n] = q^T k via PE (32 m-chunks of 128),
           P = exp(S) via ACT (groups of 3 PSUM banks), V-contraction
           accumulated in PSUM [4,512] (row 0 = denominator), then
           out = num * recip(den) + x_b, DMA rows to DRAM out [4,N].

PE Matmult supports only ONE embedded semaphore wait, so PSUM is laid out
manually in a single pool (sA banks 0-2, sB banks 3-5, vacc bank 6, ptbank
bank 7), ACT is the sole reader of stage PSUM (bn_stats reads the SBUF
copies), and tiny "gate" matmuls (1x1, reading an already-DMA'd tile or
overwriting a conflicted PSUM byte) absorb extra semaphore waits so every
real matmul needs at most one.
"""

import numpy as np

from concourse import bacc, bass, bass_utils
from concourse import tile
from concourse.bass import mybir

FP = mybir.dt.float32
AF = mybir.ActivationFunctionType
EPS = 1e-5
N = 4096
CH = 512                 # free-dim chunk for stage matmuls and attention n-chunks
NCH = N // CH            # 8
MCH = 128                # attention m-chunk (partition dim of S tiles)
NM = N // MCH            # 32
GRP = 3                  # m-chunks (PSUM banks) per exp group
NGRP = (NM + GRP - 1) // GRP  # 11 (10 groups of 3 + 1 of 2)
NQ = 67                  # padded qkv rows: q 0:16, k 32:48, v 64:67
ts = bass.ts


def build_kernel():
    # Bacc.finalize() runs generate_event_semaphores, which splits sync
    # waits to satisfy the TRN2 1-wait-per-instruction constraint.
    nc = bacc.Bacc(num_devices=8)

    x = nc.dram_tensor("x", [3, N], FP, kind="ExternalInput")
    x4 = nc.dram_tensor("x4", [4, N], FP, kind="ExternalInput")
    w1t = nc.dram_tensor("w1t", [3, 64], FP, kind="ExternalInput")
    w2t = nc.dram_tensor("w2t", [64, 128], FP, kind="ExternalInput")
    wqkvt = nc.dram_tensor("wqkvt", [128, NQ], FP, kind="ExternalInput")
    g1b1 = nc.dram_tensor("g1b1", [64, 2], FP, kind="ExternalInput")
    gbq = nc.dram_tensor("gbq", [NQ, 2], FP, kind="ExternalInput")
    eye3 = nc.dram_tensor("eye3", [3, 3], FP, kind="ExternalInput")
    out = nc.dram_tensor("out", [4, N], FP, kind="ExternalOutput")

    with tile.TileContext(nc) as tc:
        with (
            tc.tile_pool(name="const", bufs=1) as cp,
            tc.tile_pool(name="big", bufs=1) as bp,
            tc.tile_pool(name="stat", bufs=1) as sp,
            tc.tile_pool(name="pexp", bufs=2) as pexp,
            tc.tile_pool(name="tailp", bufs=2) as tailp,
            tc.tile_pool(name="ps", bufs=1, space=bass.MemorySpace.PSUM) as ps,
            tc.tile_pool(name="dram", bufs=1, space="DRAM") as dp,
        ):
            # whole-kernel PSUM layout (8 banks of [128, 512] fp32)
            sA = ps.tile([128, 3 * CH], FP)      # banks 0-2
            sB = ps.tile([128, 3 * CH], FP)      # banks 3-5
            vacc = ps.tile([4, CH], FP)          # bank 6
            ptbank = ps.tile([128, CH], FP)      # bank 7

            x_sb = cp.tile([3, N], FP)
            x4_sb = cp.tile([4, N], FP)
            w1t_sb = cp.tile([3, 64], FP)
            w2t_sb = cp.tile([64, 128], FP)
            wqkvt_sb = cp.tile([128, NQ], FP)
            g1b1_sb = cp.tile([64, 2], FP)
            gbq_sb = cp.tile([NQ, 2], FP)
            eye3_sb = cp.tile([3, 3], FP)
            ones4_sb = cp.tile([1, 4], FP)

            h_sb = bp.tile([64, N], FP)
            hpos_sb = bp.tile([64, N], FP)
            hneg_sb = bp.tile([64, N], FP)
            hact_sb = bp.tile([64, N], FP)
            feat_sb = bp.tile([128, N], FP)
            y_sb = bp.tile([NQ, N], FP)
            qkv_sb = bp.tile([NQ, N], FP)
            k_sb = bp.tile([16, N], FP)
            v_sb = bp.tile([3, N], FP)
            vt_sb = bp.tile([128, NM * 4], FP)

            nc.sync.dma_start(x_sb[:], x[:])
            nc.sync.dma_start(x4_sb[:], x4[:])
            nc.sync.dma_start(w1t_sb[:], w1t[:])
            nc.sync.dma_start(w2t_sb[:], w2t[:])
            nc.sync.dma_start(wqkvt_sb[:], wqkvt[:])
            nc.sync.dma_start(g1b1_sb[:], g1b1[:])
            nc.sync.dma_start(gbq_sb[:], gbq[:])
            nc.sync.dma_start(eye3_sb[:], eye3[:])
            nc.vector.memset(ones4_sb[:], 1.0)

            def gate(dst, src):
                # 1x1 matmul whose only purpose is to carry one semaphore
                # wait on PE so the waits of following matmuls get elided.
                nc.tensor.matmul(dst, src, src, start=True, stop=True)

            gdst = ptbank[0:1, 511:512]
            gate(gdst, w1t_sb[0:1, 0:1])
            gate(gdst, w2t_sb[0:1, 0:1])
            gate(gdst, wqkvt_sb[0:1, 0:1])
            gate(gdst, eye3_sb[0:1, 0:1])

            cc1_in = dp.tile([64, 2], FP)
            cc1_out = dp.tile([64, 2], FP)
            cc2_in = dp.tile([NQ, 2], FP)
            cc2_out = dp.tile([NQ, 2], FP)

            # ---------------- stage A: h + stats + AllReduce#1 ----------------
            hstat = sp.tile([64, 6 * NCH], FP)
            hstat2 = sp.tile([64, 2], FP)
            hmsq = sp.tile([64, 1], FP)
            he2 = sp.tile([64, 1], FP)
            hpay = sp.tile([64, 2], FP)
            gstat1 = sp.tile([64, 2], FP)
            hvar = sp.tile([64, 1], FP)
            hstd = sp.tile([64, 1], FP)
            hrstd = sp.tile([64, 1], FP)
            hs1 = sp.tile([64, 1], FP)
            htmp = sp.tile([64, 1], FP)
            ht1 = sp.tile([64, 1], FP)
            hs1n = sp.tile([64, 1], FP)
            ht1n = sp.tile([64, 1], FP)
            epsall = sp.tile([128, 1], FP)
            nc.vector.memset(epsall[:], EPS)

            for i in range(NCH):
                buf = sA if i % 2 == 0 else sB
                ph = buf[0:64, 0:CH]
                nc.tensor.matmul(
                    ph, w1t_sb[:], x_sb[:, ts(i, CH)], start=True, stop=True
                )
                nc.scalar.activation(h_sb[:, ts(i, CH)], ph, AF.Copy)
                nc.vector.bn_stats(hstat[:, i * 6 : (i + 1) * 6], h_sb[:, ts(i, CH)])

            nc.vector.bn_aggr(hstat2[:], hstat[:])
            nc.vector.tensor_scalar_mul(hpay[:, 0:1], hstat2[:, 0:1], 0.125)
            nc.vector.tensor_mul(hmsq[:], hstat2[:, 0:1], hstat2[:, 0:1])
            nc.vector.tensor_add(he2[:], hstat2[:, 1:2], hmsq[:])
            nc.vector.tensor_scalar_mul(hpay[:, 1:2], he2[:], 0.125)

            nc.gpsimd.dma_start(cc1_in[:], hpay[:])
            nc.gpsimd.collective_compute(
                "AllReduce",
                mybir.AluOpType.add,
                replica_groups=[list(range(8))],
                ins=[cc1_in.opt()],
                outs=[cc1_out.opt()],
            )
            nc.gpsimd.dma_start(gstat1[:], cc1_out[:])

            # ---------------- stage B: fold BN1, h_act, feat, y, AllReduce#2 ----
            nc.vector.tensor_mul(hmsq[:], gstat1[:, 0:1], gstat1[:, 0:1])
            nc.vector.tensor_sub(hvar[:], gstat1[:, 1:2], hmsq[:])
            nc.scalar.activation(hstd[:], hvar[:], AF.Sqrt, bias=epsall[0:64, :])
            nc.vector.reciprocal(hrstd[:], hstd[:])
            nc.vector.tensor_mul(hs1[:], hrstd[:], g1b1_sb[:, 0:1])
            nc.vector.tensor_mul(htmp[:], gstat1[:, 0:1], hs1[:])
            nc.vector.tensor_sub(ht1[:], g1b1_sb[:, 1:2], htmp[:])
            nc.vector.tensor_scalar_mul(hs1n[:], hs1[:], -1.0)
            nc.vector.tensor_scalar_mul(ht1n[:], ht1[:], -1.0)

            # leaky_relu(z, 0.2) = relu(z) - 0.2*relu(-z), z = s1*h + t1
            nc.scalar.activation(
                hpos_sb[:], h_sb[:], AF.Relu, bias=ht1[:], scale=hs1[:]
            )
            nc.scalar.activation(
                hneg_sb[:], h_sb[:], AF.Relu, bias=ht1n[:], scale=hs1n[:]
            )
            nc.vector.scalar_tensor_tensor(
                hact_sb[:], hneg_sb[:], -0.2, hpos_sb[:],
                mybir.AluOpType.mult, mybir.AluOpType.add,
            )

            ystat = sp.tile([NQ, 6 * NCH], FP)
            ystat2 = sp.tile([NQ, 2], FP)
            ymsq = sp.tile([NQ, 1], FP)
            ye2 = sp.tile([NQ, 1], FP)
            ypay = sp.tile([NQ, 2], FP)
            gstat2 = sp.tile([NQ, 2], FP)
            yvar = sp.tile([NQ, 1], FP)
            ystd = sp.tile([NQ, 1], FP)
            yrstd = sp.tile([NQ, 1], FP)
            ys = sp.tile([NQ, 1], FP)
            ytmp = sp.tile([NQ, 1], FP)
            yt = sp.tile([NQ, 1], FP)

            for i in range(NCH):
                buf = sA if i % 2 == 0 else sB
                pf = buf[0:128, CH : 2 * CH]
                nc.tensor.matmul(
                    pf, w2t_sb[:], hact_sb[:, ts(i, CH)], start=True, stop=True
                )
                nc.scalar.activation(feat_sb[:, ts(i, CH)], pf, AF.Copy)
                py = buf[0:NQ, 2 * CH : 3 * CH]
                nc.tensor.matmul(
                    py, wqkvt_sb[:], feat_sb[:, ts(i, CH)], start=True, stop=True
                )
                nc.scalar.activation(y_sb[:, ts(i, CH)], py, AF.Copy)
                nc.vector.bn_stats(ystat[:, i * 6 : (i + 1) * 6], y_sb[:, ts(i, CH)])

            nc.vector.bn_aggr(ystat2[:], ystat[:])
            nc.vector.tensor_scalar_mul(ypay[:, 0:1], ystat2[:, 0:1], 0.125)
            nc.vector.tensor_mul(ymsq[:], ystat2[:, 0:1], ystat2[:, 0:1])
            nc.vector.tensor_add(ye2[:], ystat2[:, 1:2], ymsq[:])
            nc.vector.tensor_scalar_mul(ypay[:, 1:2], ye2[:], 0.125)

            nc.gpsimd.dma_start(cc2_in[:], ypay[:])
            nc.gpsimd.collective_compute(
                "AllReduce",
                mybir.AluOpType.add,
                replica_groups=[list(range(8))],
                ins=[cc2_in.opt()],
                outs=[cc2_out.opt()],
            )
            nc.gpsimd.dma_start(gstat2[:], cc2_out[:])

            # ---------------- stage C: fold BN2, qkv, VT_aug ----------------
            nc.vector.tensor_mul(ymsq[:], gstat2[:, 0:1], gstat2[:, 0:1])
            nc.vector.tensor_sub(yvar[:], gstat2[:, 1:2], ymsq[:])
            nc.scalar.activation(ystd[:], yvar[:], AF.Sqrt, bias=epsall[0:NQ, :])
            nc.vector.reciprocal(yrstd[:], ystd[:])
            nc.vector.tensor_mul(ys[:], yrstd[:], gbq_sb[:, 0:1])
            nc.vector.tensor_mul(ytmp[:], gstat2[:, 0:1], ys[:])
            nc.vector.tensor_sub(yt[:], gbq_sb[:, 1:2], ytmp[:])

            nc.scalar.activation(
                qkv_sb[:], y_sb[:], AF.Relu, bias=yt[:], scale=ys[:]
            )
            # base-0 copies for PE operand base-partition alignment
            nc.sync.dma_start(k_sb[:], qkv_sb[32:48, :])
            nc.sync.dma_start(v_sb[:], qkv_sb[64:67, :])

            # VT_aug tiles [128,4] per m-chunk: col 0 = ones (denominator row),
            # cols 1:4 = v^T. Transposes write distinct ptbank columns (no WAR).
            nc.vector.memset(vt_sb[:], 1.0)
            gate(gdst, v_sb[0:1, 0:1])
            for i in range(NM):
                pt = ptbank[0:128, 3 * i : 3 * i + 3]
                nc.tensor.transpose(pt, v_sb[:, ts(i, MCH)], eye3_sb[:])
                nc.vector.tensor_copy(vt_sb[:, i * 4 + 1 : i * 4 + 4], pt)

            # ---------------- attention ----------------
            gate(gdst, k_sb[0:1, 0:1])
            gate(gdst, vt_sb[0:1, 127:128])

            gidx = 0
            for j in range(NCH):
                kap = k_sb[:, ts(j, CH)]
                if j > 0:
                    # absorb the DVE WAR (recip/outt of j-1 reading vacc)
                    gate(vacc[0:1, 0:1], ones4_sb[0:1, 0:1])
                prev = None  # (exp_tile, first_mchunk, count)
                for g in range(NGRP):
                    cnt = min(GRP, NM - g * GRP)
                    buf = sA if gidx % 2 == 0 else sB
                    gidx += 1
                    for u in range(cnt):
                        i = g * GRP + u
                        nc.tensor.matmul(
                            buf[0:128, ts(u, CH)],
                            qkv_sb[0:16, ts(i, MCH)],
                            kap,
                            start=True,
                            stop=True,
                        )
                    pe = pexp.tile([128, cnt * CH], FP)
                    nc.scalar.activation(pe[:], buf[0:128, 0 : cnt * CH], AF.Exp)
                    if prev is not None:
                        pbuf, i0, pcnt = prev
                        for u in range(pcnt):
                            i = i0 + u
                            nc.tensor.matmul(
                                vacc[:],
                                vt_sb[:, ts(i, 4)],
                                pbuf[:, ts(u, CH)],
                                start=(i == 0),
                                stop=False,
                            )
                    prev = (pe, g * GRP, cnt)
                pbuf, i0, pcnt = prev
                for u in range(pcnt):
                    i = i0 + u
                    nc.tensor.matmul(
                        vacc[:],
                        vt_sb[:, ts(i, 4)],
                        pbuf[:, ts(u, CH)],
                        start=False,
                        stop=(i == NM - 1),
                    )

                # vacc row 0 = denominator, rows 1:4 = numerator
                recip = tailp.tile([1, CH], FP)
                nc.vector.reciprocal(recip[:], vacc[0:1, :])
                nc.tensor.matmul(
                    ptbank[0:4, 0:CH], ones4_sb[:], recip[:], start=True, stop=True
                )
                rbc_sb = tailp.tile([4, CH], FP)
                nc.scalar.activation(rbc_sb[:], ptbank[0:4, 0:CH], AF.Copy)
                outt = tailp.tile([4, CH], FP)
                nc.vector.tensor_mul(outt[:], vacc[:], rbc_sb[:])
                outf = tailp.tile([4, CH], FP)
                nc.vector.tensor_add(outf[:], outt[:], x4_sb[:, ts(j, CH)])
                nc.sync.dma_start(out[:, ts(j, CH)], outf[:])

    nc.finalize()
    return nc


_NC_CACHE = None
TRACE = False
LAST_RESULTS = None


def make_in_maps(x, w_mlp1, g1, b1, w_mlp2, wq, g2, b2, wk, g3, b3, wv, g4, b4, alpha):
    a = float(np.asarray(alpha).reshape(-1)[0])
    f32 = np.float32
    w1t = np.ascontiguousarray(np.asarray(w_mlp1, f32).T)      # [3,64]
    w2t = np.ascontiguousarray(np.asarray(w_mlp2, f32).T)      # [64,128]
    wqkvt = np.zeros((128, NQ), dtype=f32)
    wqkvt[:, 0:16] = np.asarray(wq, f32).T
    wqkvt[:, 32:48] = np.asarray(wk, f32).T
    wqkvt[:, 64:67] = np.asarray(wv, f32).T
    g1b1 = np.ascontiguousarray(
        np.stack([np.asarray(g1, f32), np.asarray(b1, f32)], axis=1)
    )                                                          # [64,2]
    gbq = np.zeros((NQ, 2), dtype=f32)
    gbq[:, 0] = 1.0
    gbq[0:16, 0] = np.asarray(g2, f32)
    gbq[0:16, 1] = np.asarray(b2, f32)
    gbq[32:48, 0] = np.asarray(g3, f32)
    gbq[32:48, 1] = np.asarray(b3, f32)
    gbq[64:67, 0] = a * np.asarray(g4, f32)
    gbq[64:67, 1] = a * np.asarray(b4, f32)
    eye3 = np.eye(3, dtype=f32)
    xf = np.asarray(x, f32)

    maps = []
    for b in range(8):
        xb = np.ascontiguousarray(xf[b])
        x4 = np.zeros((4, N), dtype=f32)
        x4[1:4] = xb
        maps.append(
            {
                "x": xb,
                "x4": x4,
                "w1t": w1t,
                "w2t": w2t,
                "wqkvt": wqkvt,
                "g1b1": g1b1,
                "gbq": gbq,
                "eye3": eye3,
            }
        )
    return maps


def kernel(x, w_mlp1, g1, b1, w_mlp2, wq, g2, b2, wk, g3, b3, wv, g4, b4, alpha):
    global _NC_CACHE, LAST_RESULTS
    f32 = np.float32
    in_maps = make_in_maps(
        x, w_mlp1, g1, b1, w_mlp2, wq, g2, b2, wk, g3, b3, wv, g4, b4, alpha
    )

    if _NC_CACHE is None:
        _NC_CACHE = build_kernel()
    nc = _NC_CACHE

    res = bass_utils.run_bass_kernel_spmd(nc, in_maps, list(range(8)), trace=TRACE)
    LAST_RESULTS = res
    outs = [np.asarray(res.results[b]["out"], f32)[1:4] for b in range(8)]
    return np.stack(outs, axis=0)


if __name__ == "__main__":
    rng = np.random.default_rng(0)
    inputs = {
        "x": rng.standard_normal((8, 3, N), dtype=np.float32),
        "w_mlp1": rng.standard_normal((64, 3), dtype=np.float32) / np.sqrt(3),
        "g1": rng.uniform(0.5, 1.5, 64).astype(np.float32),
        "b1": (rng.standard_normal(64) * 0.1).astype(np.float32),
        "w_mlp2": rng.standard_normal((128, 64), dtype=np.float32) / 8,
        "wq": rng.standard_normal((16, 128), dtype=np.float32) / np.sqrt(128),
        "g2": rng.uniform(0.5, 1.5, 16).astype(np.float32),
        "b2": (rng.standard_normal(16) * 0.1).astype(np.float32),
        "wk": rng.standard_normal((16, 128), dtype=np.float32) / np.sqrt(128),
        "g3": rng.uniform(0.5, 1.5, 16).astype(np.float32),
        "b3": (rng.standard_normal(16) * 0.1).astype(np.float32),
        "wv": rng.standard_normal((3, 128), dtype=np.float32) / np.sqrt(128),
        "g4": rng.uniform(0.5, 1.5, 3).astype(np.float32),
        "b4": (rng.standard_normal(3) * 0.1).astype(np.float32),
        "alpha": rng.uniform(0.1, 1.0, 1).astype(np.float32),
    }
    out = kernel(**inputs)
    print(out.shape, out.dtype)


# revision 42
# speedup vs baseline: 2.2617x; 2.2617x over previous
"""Point spatial attention kernel for Trainium2, data-parallel over batch B=8.

Per core b (owning batch b, x_b [3,4096]):
  stage A: h = W1 @ x_b [64,4096]; local BN sufficient stats via bn_stats/bn_aggr;
           AllReduce#1 of [mean/8, E[h^2]/8] over 8 cores.
  stage B: fold BN1 affine into (s1,t1); h_act = leaky_relu via
           relu(z) - 0.2*relu(-z); feat = W2 @ h_act [128,4096];
           y = Wqkv_pad @ feat [67,4096] (q@0:16 | k@32:48 | v@64:67, padded
           so every SBUF access starts at partition 0/32/64);
           local stats of y; AllReduce#2 of [mean/8, E[y^2]/8].
  stage C: fold BN2 affine (alpha pre-folded into v rows host-side);
           qkv = Relu(s*y + t); DMA k,v to base-0 tiles; build VT_aug tiles
           [128,4] (col 0 = ones for denominator, cols 1:4 = v^T).
  attn:    per 512-col n-chunk: S[m,n] = q^T k via PE (32 m-chunks of 128),
           P = exp(S) via ACT (groups of 3 PSUM banks), V-contraction
           accumulated in PSUM [4,512] (row 0 = denominator), then
           out = num * recip(den) + x_b, DMA rows to DRAM out [4,N].

PE Matmult supports only ONE embedded semaphore wait, so PSUM is laid out
manually in a single pool (sA banks 0-2, sB banks 3-5, vacc bank 6, ptbank
bank 7), ACT is the sole reader of stage PSUM (bn_stats reads the SBUF
copies), and tiny "gate" matmuls (1x1, reading an already-DMA'd tile or
overwriting a conflicted PSUM byte) absorb extra semaphore waits so every
real matmul needs at most one.
"""

import numpy as np

from concourse import bacc, bass, bass_utils
from concourse import tile
from concourse.bass import mybir

FP = mybir.dt.float32
FR = mybir.dt.float32r
AF = mybir.ActivationFunctionType
EPS = 1e-5
N = 4096
CH = 512                 # free-dim chunk for stage matmuls and attention n-chunks
NCH = N // CH            # 8
MCH = 128                # attention m-chunk (partition dim of S tiles)
NM = N // MCH            # 32
GRP = 3                  # m-chunks (PSUM banks) per exp group
NGRP = (NM + GRP - 1) // GRP  # 11 (10 groups of 3 + 1 of 2)
NQ = 67                  # padded qkv rows: q 0:16, k 32:48, v 64:67
ts = bass.ts


def build_kernel():
    # Bacc.finalize() runs generate_event_semaphores, which splits sync
    # waits to satisfy the TRN2 1-wait-per-instruction constraint.
    nc = bacc.Bacc(num_devices=8)

    x = nc.dram_tensor("x", [3, N], FR, kind="ExternalInput")
    x4 = nc.dram_tensor("x4", [4, N], FP, kind="ExternalInput")
    w1t = nc.dram_tensor("w1t", [3, 64], FR, kind="ExternalInput")
    w2t = nc.dram_tensor("w2t", [64, 128], FR, kind="ExternalInput")
    wqkvt = nc.dram_tensor("wqkvt", [128, NQ], FR, kind="ExternalInput")
    g1b1 = nc.dram_tensor("g1b1", [64, 2], FP, kind="ExternalInput")
    gbq = nc.dram_tensor("gbq", [NQ, 2], FP, kind="ExternalInput")
    eye3 = nc.dram_tensor("eye3", [3, 3], FP, kind="ExternalInput")
    out = nc.dram_tensor("out", [4, N], FP, kind="ExternalOutput")

    with tile.TileContext(nc) as tc:
        with (
            tc.tile_pool(name="const", bufs=1) as cp,
            tc.tile_pool(name="big", bufs=1) as bp,
            tc.tile_pool(name="stat", bufs=1) as sp,
            tc.tile_pool(name="pexp", bufs=2) as pexp,
            tc.tile_pool(name="tailp", bufs=2) as tailp,
            tc.tile_pool(name="ps", bufs=1, space=bass.MemorySpace.PSUM) as ps,
            tc.tile_pool(name="dram", bufs=1, space="DRAM") as dp,
        ):
            # whole-kernel PSUM layout (8 banks of [128, 512] fp32)
            sA = ps.tile([128, 3 * CH], FP)      # banks 0-2
            sB = ps.tile([128, 3 * CH], FP)      # banks 3-5
            vacc = ps.tile([4, CH], FP)          # bank 6
            ptbank = ps.tile([128, CH], FP)      # bank 7

            x_sb = cp.tile([3, N], FR)
            x4_sb = cp.tile([4, N], FP)
            w1t_sb = cp.tile([3, 64], FR)
            w2t_sb = cp.tile([64, 128], FR)
            wqkvt_sb = cp.tile([128, NQ], FR)
            g1b1_sb = cp.tile([64, 2], FP)
            gbq_sb = cp.tile([NQ, 2], FP)
            eye3_sb = cp.tile([3, 3], FP)
            ones4_sb = cp.tile([1, 4], FP)

            h_sb = bp.tile([64, N], FP)
            hpos_sb = bp.tile([64, N], FP)
            hneg_sb = bp.tile([64, N], FP)
            hact_sb = bp.tile([64, N], FR)
            feat_sb = bp.tile([128, N], FR)
            y_sb = bp.tile([NQ, N], FP)
            qkv_sb = bp.tile([NQ, N], FR)
            k_sb = bp.tile([16, N], FR)
            v_sb = bp.tile([3, N], FP)
            vt_sb = bp.tile([128, NM * 4], FR)
            vtf_sb = bp.tile([128, NM * 4], FP)

            nc.sync.dma_start(x_sb[:], x[:])
            nc.sync.dma_start(x4_sb[:], x4[:])
            nc.sync.dma_start(w1t_sb[:], w1t[:])
            nc.sync.dma_start(w2t_sb[:], w2t[:])
            nc.sync.dma_start(wqkvt_sb[:], wqkvt[:])
            nc.sync.dma_start(g1b1_sb[:], g1b1[:])
            nc.sync.dma_start(gbq_sb[:], gbq[:])
            nc.sync.dma_start(eye3_sb[:], eye3[:])
            nc.vector.memset(ones4_sb[:], 1.0)

            def gate(dst, src):
                # 1x1 matmul whose only purpose is to carry one semaphore
                # wait on PE so the waits of following matmuls get elided.
                # Always fp32: 1x1 fp32r matmuls violate s3d3 ISA rules.
                src = src.bitcast(FP)
                nc.tensor.matmul(dst, src, src, start=True, stop=True)

            gdst = ptbank[0:1, 511:512]
            gate(gdst, w1t_sb[0:1, 0:1])
            gate(gdst, w2t_sb[0:1, 0:1])
            gate(gdst, wqkvt_sb[0:1, 0:1])
            gate(gdst, eye3_sb[0:1, 0:1])

            cc1_in = dp.tile([64, 2], FP)
            cc1_out = dp.tile([64, 2], FP)
            cc2_in = dp.tile([NQ, 2], FP)
            cc2_out = dp.tile([NQ, 2], FP)

            # ---------------- stage A: h + stats + AllReduce#1 ----------------
            hstat = sp.tile([64, 6 * NCH], FP)
            hstat2 = sp.tile([64, 2], FP)
            hmsq = sp.tile([64, 1], FP)
            he2 = sp.tile([64, 1], FP)
            hpay = sp.tile([64, 2], FP)
            gstat1 = sp.tile([64, 2], FP)
            hvar = sp.tile([64, 1], FP)
            hstd = sp.tile([64, 1], FP)
            hrstd = sp.tile([64, 1], FP)
            hs1 = sp.tile([64, 1], FP)
            htmp = sp.tile([64, 1], FP)
            ht1 = sp.tile([64, 1], FP)
            hs1n = sp.tile([64, 1], FP)
            ht1n = sp.tile([64, 1], FP)
            epsall = sp.tile([128, 1], FP)
            nc.vector.memset(epsall[:], EPS)

            for i in range(NCH):
                buf = sA if i % 2 == 0 else sB
                ph = buf[0:64, 0:CH]
                nc.tensor.matmul(
                    ph,
                    w1t_sb[:],
                    x_sb[:, ts(i, CH)],
                    start=True,
                    stop=True,
                )
                nc.scalar.activation(h_sb[:, ts(i, CH)], ph, AF.Copy)
                nc.vector.bn_stats(hstat[:, i * 6 : (i + 1) * 6], h_sb[:, ts(i, CH)])

            nc.vector.bn_aggr(hstat2[:], hstat[:])
            nc.vector.tensor_scalar_mul(hpay[:, 0:1], hstat2[:, 0:1], 0.125)
            nc.vector.tensor_mul(hmsq[:], hstat2[:, 0:1], hstat2[:, 0:1])
            nc.vector.tensor_add(he2[:], hstat2[:, 1:2], hmsq[:])
            nc.vector.tensor_scalar_mul(hpay[:, 1:2], he2[:], 0.125)

            nc.gpsimd.dma_start(cc1_in[:], hpay[:])
            nc.gpsimd.collective_compute(
                "AllReduce",
                mybir.AluOpType.add,
                replica_groups=[list(range(8))],
                ins=[cc1_in.opt()],
                outs=[cc1_out.opt()],
            )
            nc.gpsimd.dma_start(gstat1[:], cc1_out[:])

            # ---------------- stage B: fold BN1, h_act, feat, y, AllReduce#2 ----
            nc.vector.tensor_mul(hmsq[:], gstat1[:, 0:1], gstat1[:, 0:1])
            nc.vector.tensor_sub(hvar[:], gstat1[:, 1:2], hmsq[:])
            nc.scalar.activation(hstd[:], hvar[:], AF.Sqrt, bias=epsall[0:64, :])
            nc.vector.reciprocal(hrstd[:], hstd[:])
            nc.vector.tensor_mul(hs1[:], hrstd[:], g1b1_sb[:, 0:1])
            nc.vector.tensor_mul(htmp[:], gstat1[:, 0:1], hs1[:])
            nc.vector.tensor_sub(ht1[:], g1b1_sb[:, 1:2], htmp[:])
            nc.vector.tensor_scalar_mul(hs1n[:], hs1[:], -1.0)
            nc.vector.tensor_scalar_mul(ht1n[:], ht1[:], -1.0)

            # leaky_relu(z, 0.2) = relu(z) - 0.2*relu(-z), z = s1*h + t1
            nc.scalar.activation(
                hpos_sb[:], h_sb[:], AF.Relu, bias=ht1[:], scale=hs1[:]
            )
            nc.scalar.activation(
                hneg_sb[:], h_sb[:], AF.Relu, bias=ht1n[:], scale=hs1n[:]
            )
            # h_sb is dead after the two Relus; reuse as fp32 staging, then
            # round to fp32r via ACT (the only engine codegen accepts as a
            # float32r producer besides same-dtype DMA).
            nc.vector.scalar_tensor_tensor(
                h_sb[:], hneg_sb[:], -0.2, hpos_sb[:],
                mybir.AluOpType.mult, mybir.AluOpType.add,
            )
            nc.scalar.activation(hact_sb[:], h_sb[:], AF.Copy)

            ystat = sp.tile([NQ, 6 * NCH], FP)
            ystat2 = sp.tile([NQ, 2], FP)
            ymsq = sp.tile([NQ, 1], FP)
            ye2 = sp.tile([NQ, 1], FP)
            ypay = sp.tile([NQ, 2], FP)
            gstat2 = sp.tile([NQ, 2], FP)
            yvar = sp.tile([NQ, 1], FP)
            ystd = sp.tile([NQ, 1], FP)
            yrstd = sp.tile([NQ, 1], FP)
            ys = sp.tile([NQ, 1], FP)
            ytmp = sp.tile([NQ, 1], FP)
            yt = sp.tile([NQ, 1], FP)

            for i in range(NCH):
                buf = sA if i % 2 == 0 else sB
                pf = buf[0:128, CH : 2 * CH]
                nc.tensor.matmul(
                    pf,
                    w2t_sb[:],
                    hact_sb[:, ts(i, CH)],
                    start=True,
                    stop=True,
                )
                nc.scalar.activation(feat_sb[:, ts(i, CH)], pf, AF.Copy)
                py = buf[0:NQ, 2 * CH : 3 * CH]
                nc.tensor.matmul(
                    py,
                    wqkvt_sb[:],
                    feat_sb[:, ts(i, CH)],
                    start=True,
                    stop=True,
                )
                nc.scalar.activation(y_sb[:, ts(i, CH)], py, AF.Copy)
                nc.vector.bn_stats(ystat[:, i * 6 : (i + 1) * 6], y_sb[:, ts(i, CH)])

            nc.vector.bn_aggr(ystat2[:], ystat[:])
            nc.vector.tensor_scalar_mul(ypay[:, 0:1], ystat2[:, 0:1], 0.125)
            nc.vector.tensor_mul(ymsq[:], ystat2[:, 0:1], ystat2[:, 0:1])
            nc.vector.tensor_add(ye2[:], ystat2[:, 1:2], ymsq[:])
            nc.vector.tensor_scalar_mul(ypay[:, 1:2], ye2[:], 0.125)

            nc.gpsimd.dma_start(cc2_in[:], ypay[:])
            nc.gpsimd.collective_compute(
                "AllReduce",
                mybir.AluOpType.add,
                replica_groups=[list(range(8))],
                ins=[cc2_in.opt()],
                outs=[cc2_out.opt()],
            )
            nc.gpsimd.dma_start(gstat2[:], cc2_out[:])

            # ---------------- stage C: fold BN2, qkv, VT_aug ----------------
            nc.vector.tensor_mul(ymsq[:], gstat2[:, 0:1], gstat2[:, 0:1])
            nc.vector.tensor_sub(yvar[:], gstat2[:, 1:2], ymsq[:])
            nc.scalar.activation(ystd[:], yvar[:], AF.Sqrt, bias=epsall[0:NQ, :])
            nc.vector.reciprocal(yrstd[:], ystd[:])
            nc.vector.tensor_mul(ys[:], yrstd[:], gbq_sb[:, 0:1])
            nc.vector.tensor_mul(ytmp[:], gstat2[:, 0:1], ys[:])
            nc.vector.tensor_sub(yt[:], gbq_sb[:, 1:2], ytmp[:])

            nc.scalar.activation(
                qkv_sb[:], y_sb[:], AF.Relu, bias=yt[:], scale=ys[:]
            )
            # base-0 copies for PE operand base-partition alignment
            nc.sync.dma_start(k_sb[:], qkv_sb[32:48, :])
            nc.sync.dma_start(v_sb[:], qkv_sb[64:67, :].bitcast(FP))

            # VT_aug tiles [128,4] per m-chunk: col 0 = ones (denominator row),
            # cols 1:4 = v^T. Transposes write distinct ptbank columns (no WAR).
            nc.vector.memset(vtf_sb[:], 1.0)
            nc.scalar.activation(vt_sb[:], vtf_sb[:], AF.Copy)
            gate(gdst, v_sb[0:1, 0:1])
            for i in range(NM):
                pt = ptbank[0:128, 3 * i : 3 * i + 3]
                nc.tensor.transpose(pt, v_sb[:, ts(i, MCH)], eye3_sb[:])
                nc.scalar.activation(vt_sb[:, i * 4 + 1 : i * 4 + 4], pt, AF.Copy)

            # ---------------- attention ----------------
            gate(gdst, k_sb[0:1, 0:1])
            gate(gdst, vt_sb[0:1, 127:128])

            gidx = 0
            for j in range(NCH):
                kap = k_sb[:, ts(j, CH)]
                if j > 0:
                    # absorb the DVE WAR (recip/outt of j-1 reading vacc)
                    gate(vacc[0:1, 0:1], ones4_sb[0:1, 0:1])
                prev = None  # (exp_tile, first_mchunk, count)
                for g in range(NGRP):
                    cnt = min(GRP, NM - g * GRP)
                    buf = sA if gidx % 2 == 0 else sB
                    gidx += 1
                    for u in range(cnt):
                        i = g * GRP + u
                        nc.tensor.matmul(
                            buf[0:128, ts(u, CH)],
                            qkv_sb[0:16, ts(i, MCH)],
                            kap,
                            start=True,
                            stop=True,
                        )
                    pe = pexp.tile([128, cnt * CH], FR)
                    nc.scalar.activation(pe[:], buf[0:128, 0 : cnt * CH], AF.Exp)
                    if prev is not None:
                        pbuf, i0, pcnt = prev
                        for u in range(pcnt):
                            i = i0 + u
                            nc.tensor.matmul(
                                vacc[:],
                                vt_sb[:, ts(i, 4)],
                                pbuf[:, ts(u, CH)],
                                start=(i == 0),
                                stop=False,
                            )
                    prev = (pe, g * GRP, cnt)
                pbuf, i0, pcnt = prev
                for u in range(pcnt):
                    i = i0 + u
                    nc.tensor.matmul(
                        vacc[:],
                        vt_sb[:, ts(i, 4)],
                        pbuf[:, ts(u, CH)],
                        start=False,
                        stop=(i == NM - 1),
                    )

                # vacc row 0 = denominator, rows 1:4 = numerator
                recip = tailp.tile([1, CH], FP)
                nc.vector.reciprocal(recip[:], vacc[0:1, :])
                nc.tensor.matmul(
                    ptbank[0:4, 0:CH],
                    ones4_sb[:],
                    recip[:],
                    start=True,
                    stop=True,
                )
                rbc_sb = tailp.tile([4, CH], FP)
                nc.scalar.activation(rbc_sb[:], ptbank[0:4, 0:CH], AF.Copy)
                outt = tailp.tile([4, CH], FP)
                nc.vector.tensor_mul(outt[:], vacc[:], rbc_sb[:])
                outf = tailp.tile([4, CH], FP)
                nc.vector.tensor_add(outf[:], outt[:], x4_sb[:, ts(j, CH)])
                nc.sync.dma_start(out[:, ts(j, CH)], outf[:])

    nc.finalize()
    return nc


_NC_CACHE = None
TRACE = False
LAST_RESULTS = None


def make_in_maps(x, w_mlp1, g1, b1, w_mlp2, wq, g2, b2, wk, g3, b3, wv, g4, b4, alpha):
    a = float(np.asarray(alpha).reshape(-1)[0])
    f32 = np.float32
    w1t = np.ascontiguousarray(np.asarray(w_mlp1, f32).T)      # [3,64]
    w2t = np.ascontiguousarray(np.asarray(w_mlp2, f32).T)      # [64,128]
    wqkvt = np.zeros((128, NQ), dtype=f32)
    wqkvt[:, 0:16] = np.asarray(wq, f32).T
    wqkvt[:, 32:48] = np.asarray(wk, f32).T
    wqkvt[:, 64:67] = np.asarray(wv, f32).T
    g1b1 = np.ascontiguousarray(
        np.stack([np.asarray(g1, f32), np.asarray(b1, f32)], axis=1)
    )                                                          # [64,2]
    gbq = np.zeros((NQ, 2), dtype=f32)
    gbq[:, 0] = 1.0
    gbq[0:16, 0] = np.asarray(g2, f32)
    gbq[0:16, 1] = np.asarray(b2, f32)
    gbq[32:48, 0] = np.asarray(g3, f32)
    gbq[32:48, 1] = np.asarray(b3, f32)
    gbq[64:67, 0] = a * np.asarray(g4, f32)
    gbq[64:67, 1] = a * np.asarray(b4, f32)
    eye3 = np.eye(3, dtype=f32)
    xf = np.asarray(x, f32)

    maps = []
    for b in range(8):
        xb = np.ascontiguousarray(xf[b])
        x4 = np.zeros((4, N), dtype=f32)
        x4[1:4] = xb
        maps.append(
            {
                "x": xb,
                "x4": x4,
                "w1t": w1t,
                "w2t": w2t,
                "wqkvt": wqkvt,
                "g1b1": g1b1,
                "gbq": gbq,
                "eye3": eye3,
            }
        )
    return maps


def kernel(x, w_mlp1, g1, b1, w_mlp2, wq, g2, b2, wk, g3, b3, wv, g4, b4, alpha):
    global _NC_CACHE, LAST_RESULTS
    f32 = np.float32
    in_maps = make_in_maps(
        x, w_mlp1, g1, b1, w_mlp2, wq, g2, b2, wk, g3, b3, wv, g4, b4, alpha
    )

    if _NC_CACHE is None:
        _NC_CACHE = build_kernel()
    nc = _NC_CACHE

    res = bass_utils.run_bass_kernel_spmd(nc, in_maps, list(range(8)), trace=TRACE)
    LAST_RESULTS = res
    outs = [np.asarray(res.results[b]["out"], f32)[1:4] for b in range(8)]
    return np.stack(outs, axis=0)


if __name__ == "__main__":
    rng = np.random.default_rng(0)
    inputs = {
        "x": rng.standard_normal((8, 3, N), dtype=np.float32),
        "w_mlp1": rng.standard_normal((64, 3), dtype=np.float32) / np.sqrt(3),
        "g1": rng.uniform(0.5, 1.5, 64).astype(np.float32),
        "b1": (rng.standard_normal(64) * 0.1).astype(np.float32),
        "w_mlp2": rng.standard_normal((128, 64), dtype=np.float32) / 8,
        "wq": rng.standard_normal((16, 128), dtype=np.float32) / np.sqrt(128),
        "g2": rng.uniform(0.5, 1.5, 16).astype(np.float32),
        "b2": (rng.standard_normal(16) * 0.1).astype(np.float32),
        "wk": rng.standard_normal((16, 128), dtype=np.float32) / np.sqrt(128),
        "g3": rng.uniform(0.5, 1.5, 16).astype(np.float32),
        "b3": (rng.standard_normal(16) * 0.1).astype(np.float32),
        "wv": rng.standard_normal((3, 128), dtype=np.float32) / np.sqrt(128),
        "g4": rng.uniform(0.5, 1.5, 3).astype(np.float32),
        "b4": (rng.standard_normal(3) * 0.1).astype(np.float32),
        "alpha": rng.uniform(0.1, 1.0, 1).astype(np.float32),
    }
    out = kernel(**inputs)
    print(out.shape, out.dtype)
